# revision 4
# baseline (speedup 1.0000x reference)
"""GAT (2-layer, 8-head) Trainium2 Bass kernel, SPMD over 8 NeuronCores. V2.

V2 over the baseline:
  - Projection is NODE-SHARDED: each core projects only its 1250 owned nodes
    (1/8 of the matmul FLOPs + 1/8 of the table write traffic) from a per-core
    input slice, then an AllGather with a SHARED-memory output materializes
    the full node table ONCE in HBM (instead of 8 private 23MB copies).
  - Layer-2 projection reads the core-local emb_loc, so the emb AllGather of
    the baseline is gone; only the two table AllGathers remain.
  - The dst-side attention scores (si) are kept in SBUF for the owned nodes
    and mapped to edges with a transposed one-hot matmul, eliminating the
    256B-row sit gathers (~28us/layer of DMA) and both sit tables.
  - Graph-sum partials unchanged; final LayerNorm+MLP on host.

Structure per core (single NEFF):
  Phase A: project owned nodes -> tloc1 rows [h (1024 bf16) | sj (8) | pad],
           si kept in SBUF (si_loc1).  AllGather tloc1 -> table1 (Shared).
  Phase B: per owned 128-dst-node tile: dma_gather rows by src, per-edge
           logits = si (ohT matmul vs SBUF si) + sj (gathered), exp, scale
           messages, one-hot matmul aggregation into PSUM (denominator =
           extra matmul columns), head-mean + ELU -> emb_loc tile.
  Phase C: project emb_loc -> tloc2 / si_loc2.  AllGather tloc2 -> table2.
  Phase D: edge phase for layer 2 -> per-core partial graph-sum [1, 128].
"""

import numpy as np
import ml_dtypes

BF16 = ml_dtypes.bfloat16

N_NODES = 10000
N_EDGES = 160000
N_FEAT = 512
HEADS = 8
HID = 128
OUT = 16

N_CORES = 8
P = 128
NLOC = N_NODES // N_CORES          # 1250 owned nodes per core
NLOC_PAD = -(-NLOC // 16) * 16     # 1264: transpose-DMA blocks need rows %16


# ----------------------------------------------------------------------------
# host-side prep
# ----------------------------------------------------------------------------

def _wrap_idx(idx_i16: np.ndarray) -> np.ndarray:
    """[n] int16 -> [128, n//16] wrapped layout for dma_gather (16-partition
    wrap, replicated 8x down the partitions)."""
    n = idx_i16.shape[0]
    assert n % 16 == 0
    w = idx_i16.reshape(n // 16, 16).T  # [16, n//16]
    return np.tile(w, (8, 1))  # [128, n//16]


def host_prep(node_features, edge_src, edge_dst, W1, a1, W2, a2):
    """Returns (in_maps, meta). meta carries static structure for the builder."""
    nloc = N_NODES // N_CORES  # 1250
    T = -(-nloc // P)  # owned tiles per core (10)

    order = np.argsort(edge_dst, kind="stable")
    src_s = edge_src[order].astype(np.int64)
    dst_s = edge_dst[order].astype(np.int64)

    # common chunk count per tile position (same across cores; program is SPMD)
    tile_cnt = np.zeros((N_CORES, T), dtype=np.int64)
    for c in range(N_CORES):
        base = c * nloc
        for t in range(T):
            n0 = base + t * P
            n1 = min(base + (t + 1) * P, (c + 1) * nloc)
            lo = np.searchsorted(dst_s, n0, side="left")
            hi = np.searchsorted(dst_s, n1, side="left")
            tile_cnt[c, t] = hi - lo
    C_t = [int(-(-tile_cnt[:, t].max() // P)) for t in range(T)]  # chunks per tile pos

    SLOTS = [c * P for c in C_t]
    tot_chunks = sum(C_t)
    tot_slots = tot_chunks * P

    in_maps = []
    xbf = np.ascontiguousarray(node_features.astype(BF16))

    # fused projection matrices
    def wcat(W, a, K):
        w = np.transpose(W, (2, 0, 1)).reshape(K, HEADS * HID)  # [K, H*D]
        si = np.einsum("hdf,hd->fh", W, a[:, :HID])  # [K, H]  (dst-side score)
        sj = np.einsum("hdf,hd->fh", W, a[:, HID:])  # [K, H]  (src-side score)
        return np.concatenate([w, sj, si], axis=1).astype(BF16)  # [K, H*D+16]

    w1c = np.ascontiguousarray(wcat(W1, a1, N_FEAT))
    w2c = np.ascontiguousarray(wcat(W2, a2, HID))

    for c in range(N_CORES):
        base = c * nloc
        src_pad = np.zeros(tot_slots, dtype=np.int64)
        dstl_pad = np.full(tot_slots, 30000.0, dtype=np.float32)
        off = 0
        for t in range(T):
            n0 = base + t * P
            n1 = min(base + (t + 1) * P, (c + 1) * nloc)
            lo = np.searchsorted(dst_s, n0, side="left")
            hi = np.searchsorted(dst_s, n1, side="left")
            k = hi - lo
            sl = SLOTS[t]
            src_pad[off:off + k] = src_s[lo:hi]
            dstl_pad[off:off + k] = (dst_s[lo:hi] - n0).astype(np.float32)
            # padding: src index 0 (valid gather), dstl 30000 (never matches)
            off += sl
        assert off == tot_slots

        # wrapped int16 gather indices, per tile concatenated along free dim
        src16 = np.concatenate(
            [_wrap_idx(src_pad[sum(SLOTS[:t]):sum(SLOTS[:t + 1])].astype(np.int16))
             for t in range(T)], axis=1)
        # dstl columns: [128, tot_chunks]; element [p, g] = dstl of slot g*128+p
        dstl_cols = np.ascontiguousarray(
            dstl_pad.reshape(tot_chunks, P).T.astype(np.float32))
        # dstl broadcast rows: [128, tot_slots] int8; [p, e] = dstl of slot e
        # (same on every partition); padding -> -1 (never equals a partition id)
        dstl_i8 = np.where(dstl_pad < P, dstl_pad, -1.0).astype(np.int8)
        dstl_bc = np.ascontiguousarray(
            np.broadcast_to(dstl_i8[None, :], (P, tot_slots)))

        xloc_pad = np.zeros((NLOC_PAD, N_FEAT), dtype=BF16)
        xloc_pad[:nloc] = xbf[base:base + nloc]
        in_maps.append({
            "xloc": xloc_pad,
            "w1c": w1c,
            "w2c": w2c,
            "src16": np.ascontiguousarray(src16),
            "dstl": dstl_cols,
            "dstl_bc": dstl_bc,
        })

    meta = {
        "T": T, "C_t": C_t, "nloc": nloc,
        "tot_chunks": tot_chunks,
    }
    return in_maps, meta


# ----------------------------------------------------------------------------
# device program
# ----------------------------------------------------------------------------

HD = HEADS * HID          # 1024
ROW = HD + P              # 1152 bf16 cols per table row  (2304 B, %256)
PCOLS = HD + 16           # projection output cols: [h (1024) | sj 8 | si 8]


def build_program(meta, debug=False, stages=5, iters=1, ablate=""):
    import concourse.bacc as bacc
    import concourse.mybir as mybir
    import concourse.tile as tile
    from concourse.library_config import mlp

    dt = mybir.dt
    Alu = mybir.AluOpType
    Act = mybir.ActivationFunctionType

    T = meta["T"]
    C_t = meta["C_t"]
    nloc = meta["nloc"]
    tot_chunks = meta["tot_chunks"]

    K1 = N_FEAT // P       # 4 contraction chunks layer 1
    RB = 512               # rows per dma-transpose block

    nc = bacc.Bacc("TRN2", num_devices=N_CORES, num_swdge_queues=2,
                   dynamic_dma_scratch_size=49152)

    xloc = nc.dram_tensor("xloc", [NLOC_PAD, N_FEAT], dt.bfloat16,
                          kind="ExternalInput")
    w1c = nc.dram_tensor("w1c", [N_FEAT, PCOLS], dt.bfloat16, kind="ExternalInput")
    w2c = nc.dram_tensor("w2c", [HID, PCOLS], dt.bfloat16, kind="ExternalInput")
    src16 = nc.dram_tensor("src16", [P, tot_chunks * 8], dt.int16, kind="ExternalInput")
    dstl = nc.dram_tensor("dstl", [P, tot_chunks], dt.float32, kind="ExternalInput")
    dstl_bc = nc.dram_tensor("dstl_bc", [P, tot_chunks * P], dt.int8,
                             kind="ExternalInput")

    out_vec = nc.dram_tensor("out_vec", [1, HID], dt.float32, kind="ExternalOutput")
    dbg = {}
    if debug:
        dbg["tab1"] = nc.dram_tensor("dbg_tab1", [P, ROW], dt.bfloat16,
                                     kind="ExternalOutput")
        dbg["embloc"] = nc.dram_tensor("dbg_embloc", [nloc, HID], dt.bfloat16,
                                       kind="ExternalOutput")

    tloc1 = nc.dram_tensor("tloc1", [nloc, ROW], dt.bfloat16)
    tloc2 = nc.dram_tensor("tloc2", [nloc, ROW], dt.bfloat16)
    table1 = nc.dram_tensor("table1", [N_NODES, ROW], dt.bfloat16,
                            addr_space="Shared")
    table2 = nc.dram_tensor("table2", [N_NODES, ROW], dt.bfloat16,
                            addr_space="Shared")
    emb_loc = nc.dram_tensor("emb_loc", [NLOC_PAD, HID], dt.bfloat16)

    with tile.TileContext(nc) as tc:
        with (
            tc.tile_pool(name="const", bufs=1) as cpool,
            tc.tile_pool(name="xtp", bufs=8) as tpool,
            tc.tile_pool(name="work", bufs=2) as wpool,
            tc.tile_pool(name="chunk", bufs=4) as kpool,
            tc.tile_pool(name="psum", bufs=2, space="PSUM") as pspool,
            tc.tile_pool(name="psE", bufs=1, space="PSUM") as psepool,
            tc.tile_pool(name="psg", bufs=1, space="PSUM") as psg,
        ):
            nc.gpsimd.load_library(mlp)

            # ---- constants ----
            iota_i = cpool.tile([P, P], dt.int32)
            nc.gpsimd.iota(iota_i[:], pattern=[[1, P]], base=0, channel_multiplier=0)
            iota_bf = cpool.tile([P, P], dt.bfloat16)
            nc.vector.tensor_copy(iota_bf[:], iota_i[:])
            iop_i = cpool.tile([P, 1], dt.int32)
            nc.gpsimd.iota(iop_i[:], pattern=[[0, 1]], base=0, channel_multiplier=1)
            iota_p = cpool.tile([P, 1], dt.float32)
            nc.vector.tensor_copy(iota_p[:], iop_i[:])
            ones_col = cpool.tile([P, 1], dt.bfloat16)
            nc.gpsimd.memset(ones_col[:], 1.0)

            w1s = cpool.tile([P, K1 * PCOLS], dt.bfloat16)
            nc.sync.dma_start(
                out=w1s[:].rearrange("p (k c) -> p k c", k=K1),
                in_=w1c[:].rearrange("(k p) c -> p k c", p=P))
            w2s = cpool.tile([P, PCOLS], dt.bfloat16)
            nc.sync.dma_start(out=w2s[:], in_=w2c[:])

            srcI = cpool.tile([P, tot_chunks * 8], dt.int16)
            nc.sync.dma_start(out=srcI[:], in_=src16[:])
            dstlS = cpool.tile([P, tot_chunks], dt.float32)
            nc.sync.dma_start(out=dstlS[:], in_=dstl[:])

            # si_loc[p, t, h]: dst-side score of owned node t*128+p, head h
            si_loc1 = cpool.tile([P, T, HEADS], dt.bfloat16)
            si_loc2 = cpool.tile([P, T, HEADS], dt.bfloat16)

            # ---------------- projection phase (owned nodes only) ----------
            def project(src_dram, wtile, K, tloc, si_loc):
                nblk = -(-nloc // RB)
                for b in range(nblk):
                    r0 = b * RB               # local row offset
                    rn = min(RB, nloc - r0)
                    rn16 = -(-rn // 16) * 16  # transpose needs rows %16
                    xT = []
                    for k in range(K):
                        t_ = tpool.tile([P, RB], dt.bfloat16, tag="xT")
                        eng = nc.sync if k % 2 == 0 else nc.scalar
                        eng.dma_start_transpose(
                            t_[:, :rn16],
                            src_dram[r0:r0 + rn16, k * P:(k + 1) * P])
                        xT.append(t_)
                    SB = RB // P
                    row = wpool.tile([P, SB, ROW], dt.bfloat16, tag="row")
                    for s in range(SB):
                        l0 = r0 + s * P       # local node index of partition 0
                        if l0 >= nloc:
                            break
                        nn = min(P, nloc - l0)
                        t_idx = l0 // P
                        ps = pspool.tile([P, PCOLS], dt.float32, tag="ps")
                        for k in range(K):
                            lhsT = xT[k][:, s * P:s * P + nn]
                            rhs = wtile[:, k * PCOLS:(k + 1) * PCOLS] if K > 1 \
                                else wtile[:]
                            st, sp = (k == 0), (k == K - 1)
                            nc.tensor.matmul(ps[:nn, 0:512], lhsT=lhsT,
                                             rhs=rhs[:, 0:512], start=st, stop=sp)
                            nc.tensor.matmul(ps[:nn, 512:1024], lhsT=lhsT,
                                             rhs=rhs[:, 512:1024], start=st, stop=sp)
                            nc.tensor.matmul(ps[:nn, 1024:PCOLS], lhsT=lhsT,
                                             rhs=rhs[:, 1024:PCOLS], start=st, stop=sp)
                        nc.vector.tensor_copy(row[:nn, s, 0:HD], ps[:nn, 0:HD])
                        # sj at cols 1024:1032 of the row
                        nc.vector.tensor_copy(row[:nn, s, HD:HD + 8],
                                              ps[:nn, HD:HD + 8])
                        # si stays in SBUF for the edge phase (owned dst nodes)
                        nc.scalar.activation(si_loc[:nn, t_idx, :],
                                             ps[:nn, HD + 8:HD + 16], Act.Copy)
                    if rn == RB:
                        nc.scalar.dma_start(
                            out=tloc[r0:r0 + RB, :].rearrange(
                                "(a p) c -> p a c", p=P),
                            in_=row[:])
                    else:
                        for s in range(SB):
                            l0 = r0 + s * P
                            if l0 >= nloc:
                                break
                            nn = min(P, nloc - l0)
                            nc.scalar.dma_start(out=tloc[l0:l0 + nn, :],
                                                in_=row[:nn, s, :])

            # ---------------- edge phase ----------------
            def edges(table, si_loc, layer):
                gps = psg.tile([1, HID], dt.float32, tag="gsum", name="gps") if layer == 2 else None
                ebuf = cpool.tile([P, T * HID], dt.bfloat16, name="ebuf") if layer == 2 else None
                for t in range(T):
                    C = C_t[t]
                    ioff = sum(C_t[:t])
                    n0t = t * P
                    nn_t = min(P, nloc - n0t)
                    G = wpool.tile([P, C, ROW], dt.bfloat16, tag="G")
                    Ch = C // 2
                    nc.gpsimd.dma_gather(
                        G[:, 0:Ch, :], table[:], srcI[:, ioff * 8:(ioff + Ch) * 8],
                        Ch * P, Ch * P, ROW, single_packet=False, queue_num=0)
                    nc.gpsimd.dma_gather(
                        G[:, Ch:C, :], table[:],
                        srcI[:, (ioff + Ch) * 8:(ioff + C) * 8],
                        (C - Ch) * P, (C - Ch) * P, ROW, single_packet=False,
                        queue_num=1)
                    # dst-side scores: ohT matmul against SBUF-resident si
                    bc = wpool.tile([P, C * P], dt.int8, tag="bc")
                    nc.sync.dma_start(out=bc[:],
                                      in_=dstl_bc[:, ioff * P:(ioff + C) * P])
                    ohT = wpool.tile([P, C * P], dt.bfloat16, tag="ohT")
                    nc.vector.tensor_scalar(out=ohT[:], in0=bc[:],
                                            scalar1=iota_p[:], scalar2=None,
                                            op0=Alu.is_equal)
                    siT = kpool.tile([P, HEADS], dt.bfloat16, tag="siT")
                    if nn_t < P:
                        nc.gpsimd.memset(siT[:], 0.0)
                    nc.vector.tensor_copy(siT[:nn_t, :], si_loc[:nn_t, t, :])
                    psE = psepool.tile([P, C, HEADS], dt.float32, tag="psE")
                    for c in range(C):
                        nc.tensor.matmul(psE[:, c, :],
                                         lhsT=ohT[:, c * P:(c + 1) * P],
                                         rhs=siT[:], start=True, stop=True)
                    ps = pspool.tile([P, PCOLS], dt.float32, tag="ps")
                    # batched attention logits for the whole tile
                    LG = wpool.tile([P, C * 8], dt.float32, tag="LG")
                    nc.vector.tensor_tensor(
                        out=LG[:].rearrange("p (c e) -> p c e", c=C),
                        in0=psE[:, :, :], in1=G[:, :, HD:HD + 8], op=Alu.add)
                    LR = wpool.tile([P, C * 8], dt.float32, tag="LR")
                    nc.vector.tensor_scalar_mul(LR[:], LG[:], 0.01)
                    MX = wpool.tile([P, C * 8], dt.float32, tag="MX")
                    nc.vector.tensor_tensor(out=MX[:], in0=LG[:], in1=LR[:],
                                            op=Alu.max)
                    EX = wpool.tile([P, C * 8], dt.float32, tag="EX")
                    nc.scalar.activation(EX[:], MX[:], Act.Exp)
                    EXbf = wpool.tile([P, C, 8], dt.bfloat16, tag="EXbf")
                    nc.vector.tensor_copy(
                        EXbf[:], EX[:].rearrange("p (c e) -> p c e", c=C))
                    for c in range(C):
                        msg = kpool.tile([P, HD], dt.bfloat16, tag="msg")
                        for h in range(HEADS):
                            if h in (3, 7):
                                nc.scalar.activation(
                                    msg[:, h * HID:(h + 1) * HID],
                                    G[:, c, h * HID:(h + 1) * HID],
                                    Act.Copy,
                                    scale=EX[:, c * 8 + h:c * 8 + h + 1])
                            else:
                                nc.vector.tensor_scalar_mul(
                                    msg[:, h * HID:(h + 1) * HID],
                                    G[:, c, h * HID:(h + 1) * HID],
                                    EX[:, c * 8 + h:c * 8 + h + 1])
                        oh = kpool.tile([P, P], dt.bfloat16, tag="oh")
                        nc.vector.tensor_scalar(
                            out=oh[:], in0=iota_bf[:],
                            scalar1=dstlS[:, ioff + c:ioff + c + 1], scalar2=None,
                            op0=Alu.is_equal)
                        st, sp = (c == 0), (c == C - 1)
                        nc.tensor.matmul(ps[:, 0:512], lhsT=oh[:],
                                         rhs=msg[:, 0:512], start=st, stop=sp)
                        nc.tensor.matmul(ps[:, 512:1024], lhsT=oh[:],
                                         rhs=msg[:, 512:1024], start=st, stop=sp)
                        nc.tensor.matmul(ps[:, 1024:1032], lhsT=oh[:],
                                         rhs=EXbf[:, c, :], start=st, stop=sp)
                    # ---- postprocess tile ----
                    den = kpool.tile([P, 8], dt.float32, tag="den")
                    nc.vector.tensor_scalar(out=den[:], in0=ps[:, 1024:1032],
                                            scalar1=float(HEADS), scalar2=1e-30,
                                            op0=Alu.mult, op1=Alu.max)
                    rec = kpool.tile([P, 8], dt.float32, tag="rec")
                    nc.vector.reciprocal(rec[:], den[:])
                    Sa = kpool.tile([P, HID], dt.float32, tag="Sa")
                    Sb = kpool.tile([P, HID], dt.float32, tag="Sb")
                    nc.vector.tensor_scalar_mul(Sa[:], ps[:, 0:HID], rec[:, 0:1])
                    for h in range(1, HEADS):
                        tmp = kpool.tile([P, HID], dt.float32, tag="tmp")
                        nc.vector.tensor_scalar_mul(
                            tmp[:], ps[:, h * HID:(h + 1) * HID], rec[:, h:h + 1])
                        a, b = (Sa, Sb) if h % 2 == 1 else (Sb, Sa)
                        nc.vector.tensor_tensor(out=b[:], in0=a[:], in1=tmp[:],
                                                op=Alu.add)
                    S = Sb if HEADS % 2 == 0 else Sa  # final sum location
                    # elu(S) = exp(min(S,0)) - 1 + max(S,0)
                    neg = kpool.tile([P, HID], dt.float32, tag="neg")
                    nc.vector.tensor_scalar_min(neg[:], S[:], 0.0)
                    en = kpool.tile([P, HID], dt.float32, tag="en")
                    nc.scalar.activation(en[:], neg[:], Act.Exp)
                    pos = kpool.tile([P, HID], dt.float32, tag="pos")
                    nc.vector.tensor_scalar_max(pos[:], S[:], 0.0)
                    eadd = kpool.tile([P, HID], dt.float32, tag="eadd")
                    nc.vector.tensor_tensor(out=eadd[:], in0=en[:], in1=pos[:],
                                            op=Alu.add)
                    if layer == 1:
                        ebf = kpool.tile([P, HID], dt.bfloat16, tag="ebf")
                        nc.vector.tensor_scalar_add(ebf[:], eadd[:], -1.0)
                        nc.sync.dma_start(out=emb_loc[n0t:n0t + nn_t, :],
                                          in_=ebf[:nn_t, :])
                    else:
                        nc.vector.tensor_scalar_add(
                            ebuf[:, t * HID:(t + 1) * HID], eadd[:], -1.0)
                if layer == 2:
                    for t in range(T):
                        nn_t = min(P, nloc - t * P)
                        nc.tensor.matmul(gps[0:1, :], lhsT=ones_col[:nn_t, :],
                                         rhs=ebuf[:nn_t, t * HID:(t + 1) * HID],
                                         start=(t == 0), stop=(t == T - 1))
                return gps

            # ---------------- main flow ----------------
            def zero_out_vec():
                z = kpool.tile([1, HID], dt.float32, tag="gout", name="z")
                nc.gpsimd.memset(z[:], 0.0)
                nc.sync.dma_start(out=out_vec[:], in_=z[:])

            def flow():
                with nc.named_scope("proj1"):
                    project(xloc, w1s, K1, tloc1, si_loc1)
                if stages >= 2:
                    if ablate != "nocoll":
                        with nc.named_scope("AG1"):
                            nc.gpsimd.collective_compute(
                                "AllGather", Alu.bypass,
                                ins=[tloc1[:]], outs=[table1[:]],
                                replica_groups=[list(range(N_CORES))])
                    if debug:
                        tb0 = wpool.tile([P, ROW], dt.bfloat16, tag="row",
                                         name="tb0")
                        nc.sync.dma_start(out=tb0[:], in_=table1[0:P, :])
                        nc.sync.dma_start(out=dbg["tab1"][:], in_=tb0[:])
                if stages >= 3:
                    with nc.named_scope("edges1"):
                        edges(table1, si_loc1, layer=1)
                    if debug:
                        for b in range(-(-nloc // P)):
                            n0 = b * P
                            nn = min(P, nloc - n0)
                            te = wpool.tile([P, HID], dt.bfloat16, tag="dbgb",
                                            name="te")
                            nc.sync.dma_start(out=te[:nn, :],
                                              in_=emb_loc[n0:n0 + nn, :])
                            nc.sync.dma_start(out=dbg["embloc"][n0:n0 + nn, :],
                                              in_=te[:nn, :])
                if stages >= 4:
                    with nc.named_scope("proj2"):
                        project(emb_loc, w2s, 1, tloc2, si_loc2)
                    if ablate != "nocoll":
                        with nc.named_scope("AG2"):
                            nc.gpsimd.collective_compute(
                                "AllGather", Alu.bypass,
                                ins=[tloc2[:]], outs=[table2[:]],
                                replica_groups=[list(range(N_CORES))])
                if stages >= 5:
                    with nc.named_scope("edges2"):
                        gps = edges(table2, si_loc2, layer=2)
                        gout = kpool.tile([1, HID], dt.float32, tag="gout")
                        nc.vector.tensor_copy(gout[:], gps[:])
                        nc.sync.dma_start(out=out_vec[:], in_=gout[:])
                else:
                    zero_out_vec()

            for _it in range(iters):
                flow()

    nc.compile()
    return nc


# ----------------------------------------------------------------------------
# top-level kernel
# ----------------------------------------------------------------------------

_CACHE = {}


def _run_device(in_maps, meta):
    from concourse.bass_utils import run_bass_kernel_spmd
    key = "prog"
    if key not in _CACHE:
        _CACHE[key] = build_program(meta)
    nc = _CACHE[key]
    res = run_bass_kernel_spmd(nc, in_maps, core_ids=list(range(N_CORES)))
    return res


def host_finish(partials, ln_g, ln_b, Wl1, bl1, Wl2, bl2, Wl3, bl3):
    g = partials.sum(axis=0) / np.float32(N_NODES)  # [HID]
    mu = g.mean()
    var = ((g - mu) ** 2).mean()
    gn = (g - mu) / np.sqrt(var + 1e-5) * ln_g + ln_b
    x = Wl1 @ gn + bl1
    x = np.maximum(x, 0.01 * x)
    x = Wl2 @ x + bl2
    x = np.maximum(x, 0.01 * x)
    x = Wl3 @ x + bl3
    return np.maximum(x, 0.0).astype(np.float32)


def kernel(node_features, edge_src, edge_dst, W1, a1, W2, a2,
           ln_g, ln_b, Wl1, bl1, Wl2, bl2, Wl3, bl3):
    node_features = np.asarray(node_features, dtype=np.float32)
    edge_src = np.asarray(edge_src, dtype=np.int32)
    edge_dst = np.asarray(edge_dst, dtype=np.int32)
    in_maps, meta = host_prep(node_features, edge_src, edge_dst,
                              np.asarray(W1, np.float32), np.asarray(a1, np.float32),
                              np.asarray(W2, np.float32), np.asarray(a2, np.float32))
    res = _run_device(in_maps, meta)
    partials = np.stack([res.results[c]["out_vec"][0] for c in range(N_CORES)])
    return host_finish(partials.astype(np.float64),
                       np.asarray(ln_g, np.float64), np.asarray(ln_b, np.float64),
                       np.asarray(Wl1, np.float64), np.asarray(bl1, np.float64),
                       np.asarray(Wl2, np.float64), np.asarray(bl2, np.float64),
                       np.asarray(Wl3, np.float64), np.asarray(bl3, np.float64))
